# revision 2
# baseline (speedup 1.0000x reference)
"""Trainium2 Bass kernel for nn_CompressiveMemory_57750130262084.

The reference computes (B=8, S=4096, DK=DV=1024):
    sigma  = elu(query) + 1                                  [B,S,DK]
    memory = einsum('bkd,bsv->bkv', swap(sigma), value)      [B,DK,DV]
    z_norm = sum_s sigma                                     [B,DK]
    out    = einsum('bsd,bkv->bsv', sigma, memory)
           / einsum('bsd,bk->bs',  sigma, z_norm)[..., None]

Every einsum uses disjoint summed subscripts, so each factorises into
outer products of independent reductions:
    memory[b,k,v]    = z_norm[b,k] * VS[b,v]      with VS[b,v] = sum_s value[b,s,v]
    retrieved[b,s,v] = rs[b,s] * Z[b] * VS[b,v]   with rs = rowsum(sigma), Z = sum_k z_norm
    denom[b,s]       = rs[b,s] * Z[b]
    out[b,s,v]       = VS[b,v]                    (exactly; query cancels)

So the kernel is a column-sum of `value` over S, broadcast over S.
Sharding: data-parallel over batch, one NeuronCore per batch element.
Per-core work: read 16 MB, reduce 4096 rows -> 1 row, write 16 MB.
Measured per-NC DMA rate with all 8 cores active: ~405-414 GB/s
(HBM-domain bound, ~810 GB/s per NC pair), so the roofline for the
serial read->write schedule is ~40+40 us plus fixed overheads.

Schedule per core (v2, this session):
  - input as 32 equal 512 KB HWDGE DMAs on the sync queue. The PE does
    ALL the reduction: per chunk, ones[128,128]^T @ chunk accumulates
    the partition-reduced colsum into PSUM (592 ns per N=512 f32 pass
    steady-state = slightly faster than the 1.25 us/chunk line rate).
    Equal small DMAs matter: PE waits on each DMA's completion sem
    (~2.5 us receipt latency), and its rate barely beats the stream,
    so one big-batch stall is never recovered; with 1-chunk DMAs the
    tail after the last input byte is receipt + 1 chunk + copy.
  - PSUM -> SBUF copy in halves (DVE + ACT in parallel); the ACT
    activation table is preloaded by a dummy scalar.copy at t=0 so the
    tail copy doesn't pay the lazy ~1.3 us ACT_TABLE_LOAD.
  - output DMAs are issued on the SCALAR HWDGE queue (qScalarDynamicHW)
    with a step-0 broadcast source AP fanning the [128,W] colsum tile
    to all 4096 rows.
  - SPLIT=2 mode halves the columns: each half is reduced and written
    independently, reads stay on the sync queue and writes on the
    scalar queue, so the SDMA engines round-robin packets between the
    two rings and the left half's 8 MB of writes overlaps the right
    half's 8 MB of reads (cost: 2 KB descriptors instead of 4 KB).
"""

import numpy as np

B, S, D = 8, 4096, 1024
P = 128                 # SBUF partitions
N_CHUNK = S // P        # 32 row-chunks of 128 rows
H = 512                 # PSUM bank width in f32 (matmul N limit)
OUT_REP = 8             # row-chunks per output DMA
N_OUT = N_CHUNK // OUT_REP

SPLIT = 1               # 1: serial read->write; 2: column-split overlap

_CACHE: dict = {}


def _build_program(split):
    import concourse.mybir as mybir
    import concourse.tile as tile
    from concourse import bacc

    f32 = mybir.dt.float32
    nc = bacc.Bacc("TRN2", target_bir_lowering=False, debug=False, num_devices=B, enable_asserts=False)
    v = nc.declare_dram_parameter("value", [S, D], f32, isOutput=False)
    o = nc.declare_dram_parameter("out", [S, D], f32, isOutput=True)

    W = D // split                      # columns per split
    mm_per_chunk = W // H if W >= H else 0
    chunks_per_dma = (512 * 1024) // (P * W * 4)   # keep DMAs at 512 KB

    with tile.TileContext(nc) as tc:
        with (
            tc.tile_pool(name="in", bufs=1) as in_pool,
            tc.tile_pool(name="ones", bufs=1) as ones_pool,
            tc.tile_pool(name="bcast", bufs=1) as bcast_pool,
            tc.tile_pool(name="warm", bufs=1) as warm_pool,
            tc.tile_pool(name="psum", bufs=1, space="PSUM") as psum_pool,
        ):
            # Preload the ACT table so the tail-time scalar.copy is cheap.
            warm = warm_pool.tile([P, 2], f32)
            nc.vector.memset(warm[:], 0.0)
            nc.scalar.copy(warm[:, 0:1], warm[:, 1:2])

            ones = ones_pool.tile([P, P], f32)
            nc.vector.memset(ones[:], 1.0)

            n_dma = N_CHUNK // chunks_per_dma
            tiles = [[None] * n_dma for _ in range(split)]

            # All input DMAs first: sync-queue FIFO = left half fully
            # ahead of right half.
            for h in range(split):
                v_h = v[:][:, h * W : (h + 1) * W]                  # [S, W]
                v_rows = v_h.rearrange("(c p) m -> c p m", p=P)     # [32][128][W]
                for ti in range(n_dma):
                    t = in_pool.tile([P, chunks_per_dma * W], f32, tag=f"in{h}_{ti}")
                    src = v_rows[ti * chunks_per_dma : (ti + 1) * chunks_per_dma].rearrange("n p m -> p n m")
                    nc.sync.dma_start(t[:].rearrange("p (n m) -> p n m", n=chunks_per_dma), src)
                    tiles[h][ti] = t

            for h in range(split):
                ps = psum_pool.tile([P, W], f32, tag=f"ps{h}")
                for ti in range(n_dma):
                    t = tiles[h][ti]
                    for n in range(chunks_per_dma):
                        c = ti * chunks_per_dma + n
                        sl = t[:, n * W : (n + 1) * W]
                        for m in range(max(mm_per_chunk, 1)):
                            w0, w1 = m * H, min((m + 1) * H, W)
                            nc.tensor.matmul(
                                ps[:, w0:w1],
                                ones[:],
                                sl[:, w0:w1],
                                start=(c == 0),
                                stop=(c == N_CHUNK - 1),
                            )

                # PSUM -> SBUF in parallel halves (DVE + ACT).
                bc = bcast_pool.tile([P, W], f32, tag=f"bc{h}")
                nc.vector.tensor_copy(bc[:, 0 : W // 2], ps[:, 0 : W // 2])
                nc.scalar.copy(bc[:, W // 2 : W], ps[:, W // 2 : W])

                # Broadcast-write this half's output on the scalar queue.
                o_h = o[:][:, h * W : (h + 1) * W]
                o_re = o_h.rearrange("(i n p) m -> i p n m", i=N_OUT, n=OUT_REP, p=P)
                src = bc[:].unsqueeze(1).to_broadcast((P, OUT_REP, W))
                for i in range(N_OUT):
                    nc.scalar.dma_start(o_re[i], src)

    nc.compile()
    return nc


def _get_program():
    key = ("nc", SPLIT)
    if key not in _CACHE:
        _CACHE[key] = _build_program(SPLIT)
    return _CACHE[key]


def kernel(query: np.ndarray, value: np.ndarray) -> np.ndarray:
    from concourse.bass_utils import run_bass_kernel_spmd

    del query  # output is exactly independent of query (see module docstring)
    value = np.ascontiguousarray(value, dtype=np.float32)
    assert value.shape == (B, S, D)

    nc = _get_program()
    in_maps = [{"value": value[b]} for b in range(B)]
    try:
        res = run_bass_kernel_spmd(nc, in_maps, list(range(B)))
    except Exception:
        # The tunneled runtime occasionally surfaces a transient
        # NRT_EXEC_UNIT_UNRECOVERABLE on the first dispatch; retry once.
        import time

        time.sleep(2.0)
        res = run_bass_kernel_spmd(nc, in_maps, list(range(B)))
    return np.stack([res.results[b]["out"] for b in range(B)], axis=0)
